# revision 5
# baseline (speedup 1.0000x reference)
"""Trainium2 Bass kernel for nn_Experts (64-expert batched LSTM cell).

Math (reference):
    gates[n,b,:] = x[b,:] @ W_ih[n].T + h0[b,:] @ W_hh[n].T + b_ih[n] + b_hh[n]
    i,f,g,o = split(gates, 4);  c' = sig(f)*c0 + sig(i)*tanh(g);  h = sig(o)*tanh(c')
    out[b, n*H+h] = h[n,b,h]            # [B, N*H] = [4096, 4096]

Distribution: expert-parallel over 8 cores; core c owns experts 8c..8c+7 and
produces the contiguous output column block out[:, c*512:(c+1)*512]. All
transposes / weight reordering / bias folding are done host-side in numpy so
the device kernel is pure matmul + activation + elementwise.

Per-core device layout (E=8 local experts, GW=E*H=512), matmul operands bf16,
activations bf16, output bf16 (host converts to fp32 and applies the final
x0.5):
  - xT    [128, 4096]  x transposed      (stationary operands for PE)
  - h0T1  [65, 4096]   h0 transposed + ones row (bias trick)
  - wx    [128, 2048]  W_ih reordered: cols = gate-type-major [i|f|o|g] x E x H
  - wh1   [65, 2048]   W_hh reordered + last row = (b_ih+b_hh) reordered
  - c0s   [128, 32, 64] c0 tiled (broadcast across experts on-chip)
  (wx/wh1 i,f,o columns and bias pre-scaled by 0.5 host-side, exact in bf16)

Engine assignment per batch tile bt (32 tiles of 128 rows), chosen from the
TimelineSim cost table (per [128,512] op: DVE tensor_scalar 194ns / tensor_
tensor 327ns / STT 594ns; ACT 0.833ns/el + ~185ns fixed; Pool TT add 1111ns):
  PE  : psum[128,2048] = xT_t.T @ wx + h0T1_t.T @ wh1      (8 matmuls)
  ACT : ONE tanh over all 2048 gate cols -> sact=[Ti|Tf|To|Tg]   (the 0.5
        pre-scale makes sig(x) = (tanh(x/2)+1)/2)                 1892ns
  DVE : Fs=Tf+1 (194), m2=Fs*c0bc (327), Is=Ti+1 (194),
        m1=Is*Tg (327), Os=To+1 (194)                             1563ns/tile
  Pool: c2q[bt%4] = m1+m2   (= 2c')                               1111ns
  ACT : per QUAD of tiles: tcq = tanh(0.5*c2q) over [128,4,512]   1892ns/quad
  DVE : h2 = Os*tcq  (= 2h, bf16)                                 327ns
  DMA : one 0.5 MB output DMA per quad
ACT is the bottleneck engine at ~75.7us busy; the tail (tcq+h2) runs 4-7
tiles behind the head so ACT never stalls on the Pool/DVE c2 chain.
"""

import numpy as np

import concourse.bass as bass
import concourse.mybir as mybir
from concourse import bacc
from concourse.bass_utils import run_bass_kernel_spmd
from concourse.tile import TileContext

B, D, H, N = 4096, 128, 64, 64
NCORES = 8
EPC = N // NCORES          # experts per core
GW = EPC * H               # 512: width of one gate-type group
FW = 4 * GW                # 2048: full gates free width per batch tile
BT = B // 128              # 32 batch tiles
F32 = mybir.dt.float32
BF16 = mybir.dt.bfloat16

_GATE_ORDER = [0, 1, 3, 2]  # reorder i,f,g,o -> i,f,o,g (sig funcs contiguous)

AF = mybir.ActivationFunctionType
ALU = mybir.AluOpType


def _build_bass() -> bass.Bass:
    nc = bacc.Bacc(None, target_bir_lowering=False, debug=False)
    xT_d = nc.dram_tensor("xT", [D, B], BF16, kind="ExternalInput")
    h0T1_d = nc.dram_tensor("h0T1", [H + 1, B], BF16, kind="ExternalInput")
    c0_d = nc.dram_tensor("c0", [B, H], BF16, kind="ExternalInput")
    wx_d = nc.dram_tensor("wx", [D, FW], BF16, kind="ExternalInput")
    wh1_d = nc.dram_tensor("wh1", [H + 1, FW], BF16, kind="ExternalInput")
    out_d = nc.dram_tensor("out", [B, GW], BF16, kind="ExternalOutput")

    with TileContext(nc) as tc:
        with (
            tc.tile_pool(name="const", bufs=1) as const_pool,
            tc.tile_pool(name="work", bufs=3) as work,
            tc.tile_pool(name="ostage", bufs=2) as ostage,
            tc.tile_pool(name="psum", bufs=2, space="PSUM") as psum_pool,
        ):
            # Const loads ordered so tile 0's i-gate matmul + tanh can start
            # after only ~380 KB: wx/wh1 split per gate group, xT/h0T1 lead
            # with a small tiles-0..3 chunk.
            xT = const_pool.tile([D, B], BF16)
            h0T1 = const_pool.tile([H + 1, B], BF16)
            c0sb = const_pool.tile([128, BT, H], BF16)
            c0_v = c0_d.ap().rearrange("(u p) c -> p u c", p=128)
            wx = const_pool.tile([D, FW], BF16)
            wh1 = const_pool.tile([H + 1, FW], BF16)
            IFO = 3 * GW
            CW0 = 512                  # batch cols for tiles 0-3
            g0 = bass.ts(0, GW)
            nc.sync.dma_start(out=wx[:, g0], in_=wx_d[:, g0])
            nc.sync.dma_start(out=wh1[:, g0], in_=wh1_d[:, g0])
            nc.sync.dma_start(out=xT[:, 0:CW0], in_=xT_d[:, 0:CW0])
            nc.sync.dma_start(out=h0T1[:, 0:CW0], in_=h0T1_d[:, 0:CW0])
            for j in (1, 2, 3):
                cols = bass.ts(j, GW)
                nc.sync.dma_start(out=wx[:, cols], in_=wx_d[:, cols])
                nc.sync.dma_start(out=wh1[:, cols], in_=wh1_d[:, cols])
            nc.sync.dma_start(out=c0sb[:, 0:4], in_=c0_v[:, 0:4])
            NCH = 4
            CW = (B - CW0) // NCH      # 896-col chunks for the rest
            for k in range(NCH):
                ksl = slice(CW0 + k * CW, CW0 + (k + 1) * CW)
                nc.sync.dma_start(out=xT[:, ksl], in_=xT_d[:, ksl])
                nc.sync.dma_start(out=h0T1[:, ksl], in_=h0T1_d[:, ksl])
            nc.sync.dma_start(out=c0sb[:, 4:BT], in_=c0_v[:, 4:BT])

            QN = 4                       # tiles per c'-tanh / output quad
            osd = {}                     # bt -> Os tile (To+1)
            c2qd = {}                    # q -> c2 quad tile
            hsd = {}                     # q -> output staging tile

            def head(bt, qn=QN):
                rows = bass.ts(bt, 128)
                psum = psum_pool.tile([128, FW], F32, name=f"ps{bt}", tag="psum")
                for j in range(4):
                    cols = bass.ts(j, GW)
                    nc.tensor.matmul(psum[:, cols], xT[:, rows], wx[:, cols],
                                     start=True, stop=False)
                    nc.tensor.matmul(psum[:, cols], h0T1[:, rows], wh1[:, cols],
                                     start=False, stop=True)

                # sact = [Ti | Tf | To | Tg]: one tanh over ALL gates
                # (i,f,o pre-scaled x0.5 host-side; sig = (T+1)/2). Tiles
                # 0-1 split per gate so ACT starts before all weight DMAs.
                sact = work.tile([128, FW], BF16, name=f"sa{bt}", tag="sact")
                if bt < 2:
                    for j in range(4):
                        cols = bass.ts(j, GW)
                        nc.scalar.activation(sact[:, cols], psum[:, cols],
                                             AF.Tanh)
                else:
                    nc.scalar.activation(sact, psum, AF.Tanh)

                # c2 = 2*c' = (Tf+1)*c0 + (Ti+1)*Tg. Shifts via 4x-mode
                # tensor_scalar adds; products + final add via 2x-mode
                # tensor_tensor, all on DVE (short latency to the tail's
                # tanh); the o-gate shift on Pool (keeps DVE slack).
                is_ = work.tile([128, GW], BF16, name=f"is{bt}", tag="is")
                nc.vector.tensor_scalar_add(is_, sact[:, 0:GW], 1.0)
                m1 = work.tile([128, GW], BF16, name=f"m1{bt}", tag="m1")
                nc.vector.tensor_tensor(m1, is_, sact[:, 3 * GW:FW], ALU.mult)
                c0bc = c0sb[:, bt].unsqueeze(1).broadcast_to([128, EPC, H])
                fs = work.tile([128, GW], BF16, name=f"fs{bt}", tag="fs")
                nc.vector.tensor_scalar_add(fs, sact[:, GW:2 * GW], 1.0)
                m2 = work.tile([128, EPC, H], BF16, name=f"m2{bt}", tag="m2")
                nc.vector.tensor_tensor(
                    m2, fs.rearrange("p (e h) -> p e h", e=EPC), c0bc, ALU.mult)
                if bt % QN == 0:
                    c2qd[bt // QN] = work.tile([128, QN, GW], BF16,
                                               name=f"c2q{bt // QN}", tag="c2q")
                nc.vector.tensor_add(c2qd[bt // QN][:, bt % QN],
                                     m1, m2.rearrange("p e h -> p (e h)"))
                os_ = work.tile([128, GW], BF16, name=f"os{bt}", tag="os",
                                bufs=10)
                nc.vector.tensor_scalar_add(os_, sact[:, 2 * GW:3 * GW], 1.0)
                osd[bt] = os_

            out_v = out_d.ap().rearrange("(u p) c -> p u c", p=128)

            def tail(q, j0, nt, flush):
                # tc = tanh(c') for nt tiles of quad q (ACT input scale
                # halves c2); h2 = (To+1)*tanh(c') = 2h, stored bf16. Host
                # applies the final x0.5 in fp32.
                c2q = c2qd[q]
                tcq = work.tile([128, nt, GW], BF16, name=f"tc{q}_{j0}",
                                tag="tcq", bufs=2)
                nc.scalar.activation(tcq, c2q[:, j0:j0 + nt], AF.Tanh,
                                     scale=0.5)
                if j0 == 0:
                    hsd[q] = ostage.tile([128, QN, GW], BF16, name=f"hs{q}",
                                         tag="hs")
                hs = hsd[q]
                for j in range(j0, j0 + nt):
                    nc.vector.tensor_tensor(hs[:, j], osd.pop(q * QN + j),
                                            tcq[:, j - j0], ALU.mult)
                if flush:
                    nc.sync.dma_start(
                        out=out_v[:, q * QN + j0:q * QN + j0 + nt],
                        in_=hs[:, j0:j0 + nt])

            NQ = BT // QN
            for bt in range(BT):
                head(bt)
                q, r = bt // QN, bt % QN
                if r == 3 and q >= 1:
                    tail(q - 1, 0, QN, flush=True)      # full quads 0..NQ-2
            # Drain: last quad split so the final ACT op and DMA are small.
            tail(NQ - 1, 0, 2, flush=True)               # tiles 28,29
            tail(NQ - 1, 2, 1, flush=True)               # tile 30
            tail(NQ - 1, 3, 1, flush=True)               # tile 31

    nc.compile()
    return nc


def _prep_in_maps(x, h0, c0, W_ih, W_hh, b_ih, b_hh):
    import ml_dtypes

    BF = ml_dtypes.bfloat16
    x = np.asarray(x, np.float32)
    h0 = np.asarray(h0, np.float32)
    c0 = np.asarray(c0, np.float32)
    W_ih = np.asarray(W_ih, np.float32)
    W_hh = np.asarray(W_hh, np.float32)
    b_ih = np.asarray(b_ih, np.float32)
    b_hh = np.asarray(b_hh, np.float32)

    xT = np.ascontiguousarray(x.T).astype(BF)                         # [128, B]
    h0T1 = np.concatenate([h0.T, np.ones((1, B), np.float32)], 0).astype(BF)
    c0b = np.ascontiguousarray(c0).astype(BF)                         # [B, 64]

    Wg = W_ih.reshape(N, 4, H, D)[:, _GATE_ORDER]                     # [n,t,h,d]
    Hg = W_hh.reshape(N, 4, H, H)[:, _GATE_ORDER]                     # [n,t,h,k]
    bg = (b_ih + b_hh).reshape(N, 4, H)[:, _GATE_ORDER]               # [n,t,h]

    in_maps = []
    for c in range(NCORES):
        sl = slice(c * EPC, (c + 1) * EPC)
        wx = Wg[sl].transpose(3, 1, 0, 2).reshape(D, FW).copy()       # [d, t*e*h]
        wh = Hg[sl].transpose(3, 1, 0, 2).reshape(H, FW)
        bias = bg[sl].transpose(1, 0, 2).reshape(1, FW)
        wh1 = np.concatenate([wh, bias], 0)                           # [65, 2048]
        # Pre-scale i,f,o gate columns (incl bias row) by 0.5 — exact in
        # bf16 — so ONE tanh over all gates yields sig(x) = (tanh(x/2)+1)/2.
        wx[:, 0:3 * GW] *= 0.5
        wh1[:, 0:3 * GW] *= 0.5
        in_maps.append({
            "xT": xT,
            "h0T1": h0T1,
            "c0": c0b,
            "wx": np.ascontiguousarray(wx).astype(BF),
            "wh1": np.ascontiguousarray(wh1).astype(BF),
        })
    return in_maps


_NC_CACHE = {}


def _run(in_maps, **kwargs):
    # Cache the built module: repeated kernel() calls then reuse both the
    # Tile-scheduled program and (via the stable nc object) the compiled
    # executable instead of rebuilding/recompiling each time.
    nc = _NC_CACHE.get("nc")
    if nc is None:
        nc = _NC_CACHE["nc"] = _build_bass()
    return run_bass_kernel_spmd(nc, in_maps, list(range(NCORES)), **kwargs)


def kernel(x, h0, c0, W_ih, W_hh, b_ih, b_hh):
    in_maps = _prep_in_maps(x, h0, c0, W_ih, W_hh, b_ih, b_hh)
    res = _run(in_maps)
    # Device output is 2h in bf16; the final x0.5 and fp32 cast are host-side.
    out = np.concatenate(
        [np.asarray(res.results[c]["out"]) for c in range(NCORES)], axis=1
    ).astype(np.float32) * 0.5
    return out, out, out


# revision 6
# speedup vs baseline: 1.0146x; 1.0146x over previous
"""Trainium2 Bass kernel for nn_Experts (64-expert batched LSTM cell).

Math (reference):
    gates[n,b,:] = x[b,:] @ W_ih[n].T + h0[b,:] @ W_hh[n].T + b_ih[n] + b_hh[n]
    i,f,g,o = split(gates, 4);  c' = sig(f)*c0 + sig(i)*tanh(g);  h = sig(o)*tanh(c')
    out[b, n*H+h] = h[n,b,h]            # [B, N*H] = [4096, 4096]

Distribution: expert-parallel over 8 cores; core c owns experts 8c..8c+7 and
produces the contiguous output column block out[:, c*512:(c+1)*512]. All
transposes / weight reordering / bias folding are done host-side in numpy so
the device kernel is pure matmul + activation + elementwise.

Per-core device layout (E=8 local experts, GW=E*H=512), matmul operands bf16,
activations bf16, output bf16 (host converts to fp32 and applies the final
x0.5):
  - xT    [128, 4096]  x transposed      (stationary operands for PE)
  - h0T1  [65, 4096]   h0 transposed + ones row (bias trick)
  - wx    [128, 2048]  W_ih reordered: cols = gate-type-major [i|f|o|g] x E x H
  - wh1   [65, 2048]   W_hh reordered + last row = (b_ih+b_hh) reordered
  - c0s   [128, 32, 64] c0 tiled (broadcast across experts on-chip)
  (wx/wh1 i,f,o columns and bias pre-scaled by 0.5 host-side, exact in bf16)

Engine assignment per batch tile bt (32 tiles of 128 rows), chosen from the
TimelineSim cost table (per [128,512] op: DVE tensor_scalar 194ns / tensor_
tensor 327ns / STT 594ns; ACT 0.833ns/el + ~185ns fixed; Pool TT add 1111ns):
  PE  : psum[128,2048] = xT_t.T @ wx + h0T1_t.T @ wh1      (8 matmuls)
  ACT : ONE tanh over all 2048 gate cols -> sact=[Ti|Tf|To|Tg]   (the 0.5
        pre-scale makes sig(x) = (tanh(x/2)+1)/2)                 1892ns
  DVE : Fs=Tf+1 (194), m2=Fs*c0bc (327), Is=Ti+1 (194),
        m1=Is*Tg (327), Os=To+1 (194)                             1563ns/tile
  Pool: c2q[bt%4] = m1+m2   (= 2c')                               1111ns
  ACT : per QUAD of tiles: tcq = tanh(0.5*c2q) over [128,4,512]   1892ns/quad
  DVE : h2 = Os*tcq  (= 2h, bf16)                                 327ns
  DMA : one 0.5 MB output DMA per quad
ACT is the bottleneck engine at ~75.7us busy; the tail (tcq+h2) runs 4-7
tiles behind the head so ACT never stalls on the Pool/DVE c2 chain.
"""

import numpy as np

import concourse.bass as bass
import concourse.mybir as mybir
from concourse import bacc
from concourse.bass_utils import run_bass_kernel_spmd
from concourse.tile import TileContext

B, D, H, N = 4096, 128, 64, 64
NCORES = 8
EPC = N // NCORES          # experts per core
GW = EPC * H               # 512: width of one gate-type group
FW = 4 * GW                # 2048: full gates free width per batch tile
BT = B // 128              # 32 batch tiles
F32 = mybir.dt.float32
BF16 = mybir.dt.bfloat16

_GATE_ORDER = [0, 1, 3, 2]  # reorder i,f,g,o -> i,f,o,g (sig funcs contiguous)

AF = mybir.ActivationFunctionType
ALU = mybir.AluOpType


def _build_bass() -> bass.Bass:
    nc = bacc.Bacc(None, target_bir_lowering=False, debug=False)
    xT_d = nc.dram_tensor("xT", [D, B], BF16, kind="ExternalInput")
    h0T1_d = nc.dram_tensor("h0T1", [H + 1, B], BF16, kind="ExternalInput")
    c0_d = nc.dram_tensor("c0", [B, H], BF16, kind="ExternalInput")
    wx_d = nc.dram_tensor("wx", [D, FW], BF16, kind="ExternalInput")
    wh1_d = nc.dram_tensor("wh1", [H + 1, FW], BF16, kind="ExternalInput")
    out_d = nc.dram_tensor("out", [B, GW], BF16, kind="ExternalOutput")

    with TileContext(nc) as tc:
        with (
            tc.tile_pool(name="const", bufs=1) as const_pool,
            tc.tile_pool(name="work", bufs=3) as work,
            tc.tile_pool(name="ostage", bufs=2) as ostage,
            tc.tile_pool(name="psum", bufs=2, space="PSUM") as psum_pool,
        ):
            # Const loads ordered so tile 0's i-gate matmul + tanh can start
            # after only ~380 KB: wx/wh1 split per gate group, xT/h0T1 lead
            # with a small tiles-0..3 chunk.
            xT = const_pool.tile([D, B], BF16)
            h0T1 = const_pool.tile([H + 1, B], BF16)
            c0sb = const_pool.tile([128, BT, H], BF16)
            c0_v = c0_d.ap().rearrange("(u p) c -> p u c", p=128)
            wx = const_pool.tile([D, FW], BF16)
            wh1 = const_pool.tile([H + 1, FW], BF16)
            IFO = 3 * GW
            CW0 = 512                  # batch cols for tiles 0-3
            g0 = bass.ts(0, GW)
            nc.sync.dma_start(out=wx[:, g0], in_=wx_d[:, g0])
            nc.sync.dma_start(out=wh1[:, g0], in_=wh1_d[:, g0])
            nc.sync.dma_start(out=xT[:, 0:CW0], in_=xT_d[:, 0:CW0])
            nc.sync.dma_start(out=h0T1[:, 0:CW0], in_=h0T1_d[:, 0:CW0])
            for j in (1, 2):
                cols = bass.ts(j, GW)
                nc.sync.dma_start(out=wx[:, cols], in_=wx_d[:, cols])
                nc.sync.dma_start(out=wh1[:, cols], in_=wh1_d[:, cols])
            nc.sync.dma_start(out=c0sb[:, 0:4], in_=c0_v[:, 0:4])
            cols = bass.ts(3, GW)
            nc.sync.dma_start(out=wx[:, cols], in_=wx_d[:, cols])
            nc.sync.dma_start(out=wh1[:, cols], in_=wh1_d[:, cols])
            NCH = 4
            CW = (B - CW0) // NCH      # 896-col chunks for the rest
            for k in range(NCH):
                ksl = slice(CW0 + k * CW, CW0 + (k + 1) * CW)
                nc.sync.dma_start(out=xT[:, ksl], in_=xT_d[:, ksl])
                nc.sync.dma_start(out=h0T1[:, ksl], in_=h0T1_d[:, ksl])
                if k == 0:
                    nc.sync.dma_start(out=c0sb[:, 4:BT], in_=c0_v[:, 4:BT])

            # PE warm-up: dummy matmuls on a memset tile while the first
            # DMAs land. Keeps PE continuously busy from ~t=0 so the p-state
            # ramp (3us to full speed) completes before the real matmuls,
            # and the real PE queue drains in strict priority order.
            wmup = const_pool.tile([128, GW], BF16)
            nc.vector.memset(wmup, 0.0)
            wps = psum_pool.tile([128, FW], F32, name="wps", tag="psum")
            for r in range(8):
                nc.tensor.matmul(wps[:, 0:GW], wmup[:, 0:128], wmup,
                                 start=True, stop=True)

            QN = 4                       # tiles per c'-tanh / output quad
            osd = {}                     # bt -> Os tile (To+1)
            c2qd = {}                    # q -> c2 quad tile
            hsd = {}                     # q -> output staging tile

            def head(bt, qn=QN):
                rows = bass.ts(bt, 128)
                psum = psum_pool.tile([128, FW], F32, name=f"ps{bt}", tag="psum")
                for j in range(4):
                    cols = bass.ts(j, GW)
                    nc.tensor.matmul(psum[:, cols], xT[:, rows], wx[:, cols],
                                     start=True, stop=False)
                    nc.tensor.matmul(psum[:, cols], h0T1[:, rows], wh1[:, cols],
                                     start=False, stop=True)

                # sact = [Ti | Tf | To | Tg]: one tanh over ALL gates
                # (i,f,o pre-scaled x0.5 host-side; sig = (T+1)/2). Tiles
                # 0-1 split per gate so ACT starts before all weight DMAs.
                sact = work.tile([128, FW], BF16, name=f"sa{bt}", tag="sact")
                if bt < 2:
                    for j in range(4):
                        cols = bass.ts(j, GW)
                        nc.scalar.activation(sact[:, cols], psum[:, cols],
                                             AF.Tanh)
                else:
                    nc.scalar.activation(sact, psum, AF.Tanh)

                # c2 = 2*c' = (Tf+1)*c0 + (Ti+1)*Tg. Shifts via 4x-mode
                # tensor_scalar adds; products + final add via 2x-mode
                # tensor_tensor, all on DVE (short latency to the tail's
                # tanh); the o-gate shift on Pool (keeps DVE slack).
                is_ = work.tile([128, GW], BF16, name=f"is{bt}", tag="is")
                nc.vector.tensor_scalar_add(is_, sact[:, 0:GW], 1.0)
                m1 = work.tile([128, GW], BF16, name=f"m1{bt}", tag="m1")
                nc.vector.tensor_tensor(m1, is_, sact[:, 3 * GW:FW], ALU.mult)
                c0bc = c0sb[:, bt].unsqueeze(1).broadcast_to([128, EPC, H])
                fs = work.tile([128, GW], BF16, name=f"fs{bt}", tag="fs")
                nc.vector.tensor_scalar_add(fs, sact[:, GW:2 * GW], 1.0)
                m2 = work.tile([128, EPC, H], BF16, name=f"m2{bt}", tag="m2")
                nc.vector.tensor_tensor(
                    m2, fs.rearrange("p (e h) -> p e h", e=EPC), c0bc, ALU.mult)
                if bt % QN == 0:
                    c2qd[bt // QN] = work.tile([128, QN, GW], BF16,
                                               name=f"c2q{bt // QN}", tag="c2q")
                nc.vector.tensor_add(c2qd[bt // QN][:, bt % QN],
                                     m1, m2.rearrange("p e h -> p (e h)"))
                os_ = work.tile([128, GW], BF16, name=f"os{bt}", tag="os",
                                bufs=10)
                nc.vector.tensor_scalar_add(os_, sact[:, 2 * GW:3 * GW], 1.0)
                osd[bt] = os_

            out_v = out_d.ap().rearrange("(u p) c -> p u c", p=128)

            def tail(q, j0, nt, flush):
                # tc = tanh(c') for nt tiles of quad q (ACT input scale
                # halves c2); h2 = (To+1)*tanh(c') = 2h, stored bf16. Host
                # applies the final x0.5 in fp32.
                c2q = c2qd[q]
                tcq = work.tile([128, nt, GW], BF16, name=f"tc{q}_{j0}",
                                tag="tcq", bufs=2)
                nc.scalar.activation(tcq, c2q[:, j0:j0 + nt], AF.Tanh,
                                     scale=0.5)
                if j0 == 0:
                    hsd[q] = ostage.tile([128, QN, GW], BF16, name=f"hs{q}",
                                         tag="hs")
                hs = hsd[q]
                for j in range(j0, j0 + nt):
                    nc.vector.tensor_tensor(hs[:, j], osd.pop(q * QN + j),
                                            tcq[:, j - j0], ALU.mult)
                if flush:
                    nc.sync.dma_start(
                        out=out_v[:, q * QN + j0:q * QN + j0 + nt],
                        in_=hs[:, j0:j0 + nt])

            NQ = BT // QN
            for bt in range(BT):
                head(bt)
                q, r = bt // QN, bt % QN
                if r == 3 and q >= 1:
                    tail(q - 1, 0, QN, flush=True)      # full quads 0..NQ-2
            # Drain: last quad split so the final ACT op and DMA are small.
            tail(NQ - 1, 0, 2, flush=True)               # tiles 28,29
            tail(NQ - 1, 2, 1, flush=True)               # tile 30
            tail(NQ - 1, 3, 1, flush=True)               # tile 31

    nc.compile()
    return nc


def _prep_in_maps(x, h0, c0, W_ih, W_hh, b_ih, b_hh):
    import ml_dtypes

    BF = ml_dtypes.bfloat16
    x = np.asarray(x, np.float32)
    h0 = np.asarray(h0, np.float32)
    c0 = np.asarray(c0, np.float32)
    W_ih = np.asarray(W_ih, np.float32)
    W_hh = np.asarray(W_hh, np.float32)
    b_ih = np.asarray(b_ih, np.float32)
    b_hh = np.asarray(b_hh, np.float32)

    xT = np.ascontiguousarray(x.T).astype(BF)                         # [128, B]
    h0T1 = np.concatenate([h0.T, np.ones((1, B), np.float32)], 0).astype(BF)
    c0b = np.ascontiguousarray(c0).astype(BF)                         # [B, 64]

    Wg = W_ih.reshape(N, 4, H, D)[:, _GATE_ORDER]                     # [n,t,h,d]
    Hg = W_hh.reshape(N, 4, H, H)[:, _GATE_ORDER]                     # [n,t,h,k]
    bg = (b_ih + b_hh).reshape(N, 4, H)[:, _GATE_ORDER]               # [n,t,h]

    in_maps = []
    for c in range(NCORES):
        sl = slice(c * EPC, (c + 1) * EPC)
        wx = Wg[sl].transpose(3, 1, 0, 2).reshape(D, FW).copy()       # [d, t*e*h]
        wh = Hg[sl].transpose(3, 1, 0, 2).reshape(H, FW)
        bias = bg[sl].transpose(1, 0, 2).reshape(1, FW)
        wh1 = np.concatenate([wh, bias], 0)                           # [65, 2048]
        # Pre-scale i,f,o gate columns (incl bias row) by 0.5 — exact in
        # bf16 — so ONE tanh over all gates yields sig(x) = (tanh(x/2)+1)/2.
        wx[:, 0:3 * GW] *= 0.5
        wh1[:, 0:3 * GW] *= 0.5
        in_maps.append({
            "xT": xT,
            "h0T1": h0T1,
            "c0": c0b,
            "wx": np.ascontiguousarray(wx).astype(BF),
            "wh1": np.ascontiguousarray(wh1).astype(BF),
        })
    return in_maps


_NC_CACHE = {}


def _run(in_maps, **kwargs):
    # Cache the built module: repeated kernel() calls then reuse both the
    # Tile-scheduled program and (via the stable nc object) the compiled
    # executable instead of rebuilding/recompiling each time.
    nc = _NC_CACHE.get("nc")
    if nc is None:
        nc = _NC_CACHE["nc"] = _build_bass()
    return run_bass_kernel_spmd(nc, in_maps, list(range(NCORES)), **kwargs)


def kernel(x, h0, c0, W_ih, W_hh, b_ih, b_hh):
    in_maps = _prep_in_maps(x, h0, c0, W_ih, W_hh, b_ih, b_hh)
    res = _run(in_maps)
    # Device output is 2h in bf16; the final x0.5 and fp32 cast are host-side.
    out = np.concatenate(
        [np.asarray(res.results[c]["out"]) for c in range(NCORES)], axis=1
    ).astype(np.float32) * 0.5
    return out, out, out


# revision 10
# speedup vs baseline: 1.1127x; 1.0967x over previous
"""Trainium2 Bass kernel for nn_Experts (64-expert batched LSTM cell).

Math (reference):
    gates[n,b,:] = x[b,:] @ W_ih[n].T + h0[b,:] @ W_hh[n].T + b_ih[n] + b_hh[n]
    i,f,g,o = split(gates, 4);  c' = sig(f)*c0 + sig(i)*tanh(g);  h = sig(o)*tanh(c')
    out[b, n*H+h] = h[n,b,h]            # [B, N*H] = [4096, 4096]

Distribution: expert-parallel over 8 cores; core c owns experts 8c..8c+7 and
produces the contiguous output column block out[:, c*512:(c+1)*512]. All
transposes / weight reordering / bias folding are done host-side in numpy so
the device kernel is pure matmul + activation + elementwise.

Per-core device layout (E=8 local experts, GW=E*H=512), matmul operands bf16,
activations bf16, output bf16 (host converts to fp32 and applies the final
x0.5):
  - xT    [128, 4096]  x transposed      (stationary operands for PE)
  - h0T1  [65, 4096]   h0 transposed + ones row (bias trick)
  - wx    [128, 2048]  W_ih reordered: cols = gate-type-major [i|f|o|g] x E x H
  - wh1   [65, 2048]   W_hh reordered + last row = (b_ih+b_hh) reordered
  - c0s   [128, 32, 64] c0 tiled (broadcast across experts on-chip)
  (wx/wh1 i,f,o columns and bias pre-scaled by 0.5 host-side, exact in bf16)

Engine assignment per batch tile bt (32 tiles of 128 rows), chosen from the
TimelineSim cost table (per [128,512] op: DVE tensor_scalar 194ns / tensor_
tensor 327ns / STT 594ns; ACT 0.833ns/el + ~185ns fixed; Pool TT add 1111ns):
  PE  : psum[128,2048] = xT_t.T @ wx + h0T1_t.T @ wh1      (8 matmuls)
  ACT : ONE tanh over all 2048 gate cols -> sact=[Ti|Tf|To|Tg]   (the 0.5
        pre-scale makes sig(x) = (tanh(x/2)+1)/2)                 1892ns
  DVE : Fs=Tf+1 (194), m2=Fs*c0bc (327), Is=Ti+1 (194),
        m1=Is*Tg (327), Os=To+1 (194)                             1563ns/tile
  Pool: c2q[bt%4] = m1+m2   (= 2c')                               1111ns
  ACT : per QUAD of tiles: tcq = tanh(0.5*c2q) over [128,4,512]   1892ns/quad
  DVE : h2 = Os*tcq  (= 2h, bf16)                                 327ns
  DMA : one 0.5 MB output DMA per quad
ACT is the bottleneck engine at ~75.7us busy; the tail (tcq+h2) runs 4-7
tiles behind the head so ACT never stalls on the Pool/DVE c2 chain.
"""

import numpy as np

import concourse.bass as bass
import concourse.mybir as mybir
from concourse import bacc
from concourse.bass_utils import run_bass_kernel_spmd
from concourse.tile import TileContext

B, D, H, N = 4096, 128, 64, 64
NCORES = 8
EPC = N // NCORES          # experts per core
GW = EPC * H               # 512: width of one gate-type group
FW = 4 * GW                # 2048: full gates free width per batch tile
BT = B // 128              # 32 batch tiles
F32 = mybir.dt.float32
BF16 = mybir.dt.bfloat16

_GATE_ORDER = [0, 1, 3, 2]  # reorder i,f,g,o -> i,f,o,g (sig funcs contiguous)

AF = mybir.ActivationFunctionType
ALU = mybir.AluOpType


def _build_bass() -> bass.Bass:
    nc = bacc.Bacc(None, target_bir_lowering=False, debug=False)
    # boot packs everything tiles 0-3 need from the i gate: one 512 KB DMA
    # instead of four small ones (each DMA costs ~625ns issue + ~900ns sem).
    # Layout: [:, 0:512]=xT[:, 0:512], [:, 512:1024]=wx i-cols,
    # [0:65, 1024:1536]=h0T1[:, 0:512], [0:65, 1536:2048]=wh1 i-cols.
    boot_d = nc.dram_tensor("boot", [D, 4 * GW], BF16, kind="ExternalInput")
    xT_d = nc.dram_tensor("xT", [D, B], BF16, kind="ExternalInput")
    h0T1_d = nc.dram_tensor("h0T1", [H + 1, B], BF16, kind="ExternalInput")
    c0_d = nc.dram_tensor("c0", [B, H], BF16, kind="ExternalInput")
    wx_d = nc.dram_tensor("wx", [D, FW], BF16, kind="ExternalInput")
    wh1_d = nc.dram_tensor("wh1", [H + 1, FW], BF16, kind="ExternalInput")
    out_d = nc.dram_tensor("out", [B, GW], BF16, kind="ExternalOutput")

    with TileContext(nc) as tc:
        with (
            tc.tile_pool(name="const", bufs=1) as const_pool,
            tc.tile_pool(name="work", bufs=3) as work,
            tc.tile_pool(name="ostage", bufs=2) as ostage,
            tc.tile_pool(name="psum", bufs=2, space="PSUM") as psum_pool,
        ):
            # Const loads: the boot DMA first (tile 0-3 x/h0 + i-gate
            # weights in one transfer), then per-gate-group weight chunks,
            # then the bulk xT/h0T1/c0.
            boot = const_pool.tile([D, 4 * GW], BF16)
            xT = const_pool.tile([D, B], BF16)
            h0T1 = const_pool.tile([H + 1, B], BF16)
            c0sb = const_pool.tile([128, BT, H], BF16)
            c0_v = c0_d.ap().rearrange("(u p) c -> p u c", p=128)
            wx = const_pool.tile([D, FW], BF16)
            wh1 = const_pool.tile([H + 1, FW], BF16)
            IFO = 3 * GW
            CW0 = 512                  # batch cols served by boot (tiles 0-3)
            nc.sync.dma_start(out=boot, in_=boot_d.ap())
            for j in (1, 2, 3):
                cols = bass.ts(j, GW)
                nc.sync.dma_start(out=wx[:, cols], in_=wx_d[:, cols])
                nc.sync.dma_start(out=wh1[:, cols], in_=wh1_d[:, cols])
            nc.sync.dma_start(out=c0sb[:, 0:4], in_=c0_v[:, 0:4])
            NCH = 4
            CW = (B - CW0) // NCH      # 896-col chunks for the rest
            for k in range(NCH):
                ksl = slice(CW0 + k * CW, CW0 + (k + 1) * CW)
                nc.sync.dma_start(out=xT[:, ksl], in_=xT_d[:, ksl])
                nc.sync.dma_start(out=h0T1[:, ksl], in_=h0T1_d[:, ksl])
                if k == 0:
                    nc.sync.dma_start(out=c0sb[:, 4:BT], in_=c0_v[:, 4:BT])

            # PE warm-up: dummy matmuls on a memset tile while the first
            # DMAs land. Keeps PE continuously busy from ~t=0 so the p-state
            # ramp (3us to full speed) completes before the real matmuls,
            # and the real PE queue drains in strict priority order.
            wmup = const_pool.tile([128, GW], BF16)
            nc.vector.memset(wmup, 0.0)
            wps = psum_pool.tile([128, FW], F32, name="wps", tag="psum")
            for r in range(8):
                nc.tensor.matmul(wps[:, 0:GW], wmup[:, 0:128], wmup,
                                 start=True, stop=True)

            def mm_operands(bt, j):
                # (lhsT_x, rhs_x, lhsT_h, rhs_h) for gate group j of tile bt;
                # tiles 0-3 and the i gate group read from the boot tile.
                rows = bass.ts(bt, 128)
                if bt < 4:
                    lx = boot[:, bt * 128:(bt + 1) * 128]
                    lh = boot[0:H + 1, 2 * GW + bt * 128:2 * GW + (bt + 1) * 128]
                else:
                    lx = xT[:, rows]
                    lh = h0T1[:, rows]
                if j == 0:
                    return lx, boot[:, GW:2 * GW], lh, boot[0:H + 1, 3 * GW:FW]
                cols = bass.ts(j, GW)
                return lx, wx[:, cols], lh, wh1[:, cols]

            QN = 4                       # tiles per c'-tanh / output quad
            osd = {}                     # bt -> Os tile (To+1)
            c2qd = {}                    # q -> c2 quad tile
            hsd = {}                     # q -> output staging tile

            def head(bt, qn=QN):
                psum = psum_pool.tile([128, FW], F32, name=f"ps{bt}", tag="psum")
                # sact = [Ti | Tf | To | Tg]: one tanh over ALL gates
                # (i,f,o pre-scaled x0.5 host-side; sig = (T+1)/2). Tiles
                # 0-1 interleave matmul-pair + per-gate tanh EMISSION so the
                # tanh's coalesced wait only covers that gate's matmuls and
                # ACT starts before the later weight DMAs land.
                sact = work.tile([128, FW], BF16, name=f"sa{bt}", tag="sact")
                if bt < 2:
                    for j in range(4):
                        cols = bass.ts(j, GW)
                        lx, rx, lh, rh = mm_operands(bt, j)
                        nc.tensor.matmul(psum[:, cols], lx, rx,
                                         start=True, stop=False)
                        nc.tensor.matmul(psum[:, cols], lh, rh,
                                         start=False, stop=True)
                        nc.scalar.activation(sact[:, cols], psum[:, cols],
                                             AF.Tanh)
                else:
                    for j in range(4):
                        cols = bass.ts(j, GW)
                        lx, rx, lh, rh = mm_operands(bt, j)
                        nc.tensor.matmul(psum[:, cols], lx, rx,
                                         start=True, stop=False)
                        nc.tensor.matmul(psum[:, cols], lh, rh,
                                         start=False, stop=True)
                    nc.scalar.activation(sact, psum, AF.Tanh)

                # c2 = 2*c' = (Tf+1)*c0 + (Ti+1)*Tg. Shifts via 4x-mode
                # tensor_scalar adds; products + final add via 2x-mode
                # tensor_tensor, all on DVE (short latency to the tail's
                # tanh); the o-gate shift on Pool (keeps DVE slack).
                is_ = work.tile([128, GW], BF16, name=f"is{bt}", tag="is")
                nc.vector.tensor_scalar_add(is_, sact[:, 0:GW], 1.0)
                m1 = work.tile([128, GW], BF16, name=f"m1{bt}", tag="m1")
                nc.vector.tensor_tensor(m1, is_, sact[:, 3 * GW:FW], ALU.mult)
                c0bc = c0sb[:, bt].unsqueeze(1).broadcast_to([128, EPC, H])
                fs = work.tile([128, GW], BF16, name=f"fs{bt}", tag="fs")
                nc.vector.tensor_scalar_add(fs, sact[:, GW:2 * GW], 1.0)
                m2 = work.tile([128, EPC, H], BF16, name=f"m2{bt}", tag="m2")
                nc.vector.tensor_tensor(
                    m2, fs.rearrange("p (e h) -> p e h", e=EPC), c0bc, ALU.mult)
                if bt % QN == 0:
                    c2qd[bt // QN] = work.tile([128, QN, GW], BF16,
                                               name=f"c2q{bt // QN}", tag="c2q")
                nc.vector.tensor_add(c2qd[bt // QN][:, bt % QN],
                                     m1, m2.rearrange("p e h -> p (e h)"))
                os_ = work.tile([128, GW], BF16, name=f"os{bt}", tag="os",
                                bufs=10)
                nc.vector.tensor_scalar_add(os_, sact[:, 2 * GW:3 * GW], 1.0)
                osd[bt] = os_

            out_v = out_d.ap().rearrange("(u p) c -> p u c", p=128)

            def tail(q, j0, nt, flush):
                # tc = tanh(c') for nt tiles of quad q (ACT input scale
                # halves c2); h2 = (To+1)*tanh(c') = 2h, stored bf16. Host
                # applies the final x0.5 in fp32.
                c2q = c2qd[q]
                tcq = work.tile([128, nt, GW], BF16, name=f"tc{q}_{j0}",
                                tag="tcq", bufs=2)
                nc.scalar.activation(tcq, c2q[:, j0:j0 + nt], AF.Tanh,
                                     scale=0.5)
                if j0 == 0:
                    hsd[q] = ostage.tile([128, QN, GW], BF16, name=f"hs{q}",
                                         tag="hs")
                hs = hsd[q]
                for j in range(j0, j0 + nt):
                    nc.vector.tensor_tensor(hs[:, j], osd.pop(q * QN + j),
                                            tcq[:, j - j0], ALU.mult)
                if flush:
                    nc.sync.dma_start(
                        out=out_v[:, q * QN + j0:q * QN + j0 + nt],
                        in_=hs[:, j0:j0 + nt])

            NQ = BT // QN
            for bt in range(BT):
                head(bt)
                q, r = bt // QN, bt % QN
                if r == 3 and q >= 1:
                    tail(q - 1, 0, QN, flush=True)      # full quads 0..NQ-2
            # Drain: last quad split so the final ACT op and DMA are small.
            tail(NQ - 1, 0, 2, flush=True)               # tiles 28,29
            tail(NQ - 1, 2, 1, flush=True)               # tile 30
            tail(NQ - 1, 3, 1, flush=True)               # tile 31

    nc.compile()
    return nc


def _prep_in_maps(x, h0, c0, W_ih, W_hh, b_ih, b_hh):
    import ml_dtypes

    BF = ml_dtypes.bfloat16
    x = np.asarray(x, np.float32)
    h0 = np.asarray(h0, np.float32)
    c0 = np.asarray(c0, np.float32)
    W_ih = np.asarray(W_ih, np.float32)
    W_hh = np.asarray(W_hh, np.float32)
    b_ih = np.asarray(b_ih, np.float32)
    b_hh = np.asarray(b_hh, np.float32)

    xT = np.ascontiguousarray(x.T).astype(BF)                         # [128, B]
    h0T1 = np.concatenate([h0.T, np.ones((1, B), np.float32)], 0).astype(BF)
    c0b = np.ascontiguousarray(c0).astype(BF)                         # [B, 64]

    Wg = W_ih.reshape(N, 4, H, D)[:, _GATE_ORDER]                     # [n,t,h,d]
    Hg = W_hh.reshape(N, 4, H, H)[:, _GATE_ORDER]                     # [n,t,h,k]
    bg = (b_ih + b_hh).reshape(N, 4, H)[:, _GATE_ORDER]               # [n,t,h]

    in_maps = []
    for c in range(NCORES):
        sl = slice(c * EPC, (c + 1) * EPC)
        wx = Wg[sl].transpose(3, 1, 0, 2).reshape(D, FW).copy()       # [d, t*e*h]
        wh = Hg[sl].transpose(3, 1, 0, 2).reshape(H, FW)
        bias = bg[sl].transpose(1, 0, 2).reshape(1, FW)
        wh1 = np.concatenate([wh, bias], 0)                           # [65, 2048]
        # Pre-scale i,f,o gate columns (incl bias row) by 0.5 — exact in
        # bf16 — so ONE tanh over all gates yields sig(x) = (tanh(x/2)+1)/2.
        wx[:, 0:3 * GW] *= 0.5
        wh1[:, 0:3 * GW] *= 0.5
        wxb = np.ascontiguousarray(wx).astype(BF)
        wh1b = np.ascontiguousarray(wh1).astype(BF)
        # boot: tiles 0-3 x/h0 + the i-gate weights, one contiguous block.
        boot = np.zeros((D, FW), BF)
        boot[:, 0:GW] = xT[:, 0:GW]
        boot[:, GW:2 * GW] = wxb[:, 0:GW]
        boot[0:H + 1, 2 * GW:3 * GW] = h0T1[:, 0:GW]
        boot[0:H + 1, 3 * GW:FW] = wh1b[:, 0:GW]
        in_maps.append({
            "boot": boot,
            "xT": xT,
            "h0T1": h0T1,
            "c0": c0b,
            "wx": wxb,
            "wh1": wh1b,
        })
    return in_maps


_NC_CACHE = {}


def _run(in_maps, **kwargs):
    # Cache the built module: repeated kernel() calls then reuse both the
    # Tile-scheduled program and (via the stable nc object) the compiled
    # executable instead of rebuilding/recompiling each time.
    nc = _NC_CACHE.get("nc")
    if nc is None:
        nc = _NC_CACHE["nc"] = _build_bass()
    return run_bass_kernel_spmd(nc, in_maps, list(range(NCORES)), **kwargs)


def kernel(x, h0, c0, W_ih, W_hh, b_ih, b_hh):
    in_maps = _prep_in_maps(x, h0, c0, W_ih, W_hh, b_ih, b_hh)
    res = _run(in_maps)
    # Device output is 2h in bf16; the final x0.5 and fp32 cast are host-side.
    out = np.concatenate(
        [np.asarray(res.results[c]["out"]) for c in range(NCORES)], axis=1
    ).astype(np.float32) * 0.5
    return out, out, out


# revision 14
# speedup vs baseline: 1.1188x; 1.0054x over previous
"""Trainium2 Bass kernel for nn_Experts (64-expert batched LSTM cell).

Math (reference):
    gates[n,b,:] = x[b,:] @ W_ih[n].T + h0[b,:] @ W_hh[n].T + b_ih[n] + b_hh[n]
    i,f,g,o = split(gates, 4);  c' = sig(f)*c0 + sig(i)*tanh(g);  h = sig(o)*tanh(c')
    out[b, n*H+h] = h[n,b,h]            # [B, N*H] = [4096, 4096]

Distribution: expert-parallel over 8 cores; core c owns experts 8c..8c+7 and
produces the contiguous output column block out[:, c*512:(c+1)*512]. All
transposes / weight reordering / bias folding are done host-side in numpy so
the device kernel is pure matmul + activation + elementwise.

Per-core device layout (E=8 local experts, GW=E*H=512), matmul operands bf16,
activations bf16, output bf16 (host converts to fp32 and applies the final
x0.5):
  - xT    [128, 4096]  x transposed      (stationary operands for PE)
  - h0T1  [65, 4096]   h0 transposed + ones row (bias trick)
  - wx    [128, 2048]  W_ih reordered: cols = gate-type-major [i|f|o|g] x E x H
  - wh1   [65, 2048]   W_hh reordered + last row = (b_ih+b_hh) reordered
  - c0s   [128, 32, 64] c0 tiled (broadcast across experts on-chip)
  (wx/wh1 i,f,o columns and bias pre-scaled by 0.5 host-side, exact in bf16)

Engine assignment per batch tile bt (32 tiles of 128 rows), chosen from the
TimelineSim cost table (per [128,512] op: DVE tensor_scalar 194ns / tensor_
tensor 327ns / STT 594ns; ACT 0.833ns/el + ~185ns fixed; Pool TT add 1111ns):
  PE  : psum[128,2048] = xT_t.T @ wx + h0T1_t.T @ wh1      (8 matmuls)
  ACT : ONE tanh over all 2048 gate cols -> sact=[Ti|Tf|To|Tg]   (the 0.5
        pre-scale makes sig(x) = (tanh(x/2)+1)/2)                 1892ns
  DVE : Fs=Tf+1 (194), m2=Fs*c0bc (327), Is=Ti+1 (194),
        m1=Is*Tg (327), Os=To+1 (194)                             1563ns/tile
  Pool: c2q[bt%4] = m1+m2   (= 2c')                               1111ns
  ACT : per QUAD of tiles: tcq = tanh(0.5*c2q) over [128,4,512]   1892ns/quad
  DVE : h2 = Os*tcq  (= 2h, bf16)                                 327ns
  DMA : one 0.5 MB output DMA per quad
ACT is the bottleneck engine at ~75.7us busy; the tail (tcq+h2) runs 4-7
tiles behind the head so ACT never stalls on the Pool/DVE c2 chain.
"""

import numpy as np

import concourse.bass as bass
import concourse.mybir as mybir
from concourse import bacc
from concourse.bass_utils import run_bass_kernel_spmd
from concourse.tile import TileContext

B, D, H, N = 4096, 128, 64, 64
NCORES = 8
EPC = N // NCORES          # experts per core
GW = EPC * H               # 512: width of one gate-type group
FW = 4 * GW                # 2048: full gates free width per batch tile
BT = B // 128              # 32 batch tiles
F32 = mybir.dt.float32
BF16 = mybir.dt.bfloat16

_GATE_ORDER = [0, 1, 3, 2]  # reorder i,f,g,o -> i,f,o,g (sig funcs contiguous)

AF = mybir.ActivationFunctionType
ALU = mybir.AluOpType


def _build_bass() -> bass.Bass:
    nc = bacc.Bacc(None, target_bir_lowering=False, debug=False)
    # boot packs everything tiles 0-3 need from the i gate: one 512 KB DMA
    # instead of four small ones (each DMA costs ~625ns issue + ~900ns sem).
    # Layout: [:, 0:512]=xT[:, 0:512], [:, 512:1024]=wx i-cols,
    # [0:65, 1024:1536]=h0T1[:, 0:512], [0:65, 1536:2048]=wh1 i-cols.
    boot_d = nc.dram_tensor("boot", [D, 4 * GW], BF16, kind="ExternalInput")
    xT_d = nc.dram_tensor("xT", [D, B], BF16, kind="ExternalInput")
    h0T1_d = nc.dram_tensor("h0T1", [H + 1, B], BF16, kind="ExternalInput")
    c0_d = nc.dram_tensor("c0", [B, H], BF16, kind="ExternalInput")
    wx_d = nc.dram_tensor("wx", [D, FW], BF16, kind="ExternalInput")
    wh1_d = nc.dram_tensor("wh1", [H + 1, FW], BF16, kind="ExternalInput")
    out_d = nc.dram_tensor("out", [B, GW], BF16, kind="ExternalOutput")

    with TileContext(nc) as tc:
        with (
            tc.tile_pool(name="const", bufs=1) as const_pool,
            tc.tile_pool(name="work", bufs=3) as work,
            tc.tile_pool(name="ostage", bufs=2) as ostage,
            tc.tile_pool(name="psum", bufs=2, space="PSUM") as psum_pool,
        ):
            # Const loads: the boot DMA first (tile 0-3 x/h0 + i-gate
            # weights in one transfer), then per-gate-group weight chunks,
            # then the bulk xT/h0T1/c0.
            boot = const_pool.tile([D, 4 * GW], BF16)
            xT = const_pool.tile([D, B], BF16)
            h0T1 = const_pool.tile([H + 1, B], BF16)
            c0sb = const_pool.tile([128, BT, H], BF16)
            c0_v = c0_d.ap().rearrange("(u p) c -> p u c", p=128)
            wx = const_pool.tile([D, FW], BF16)
            wh1 = const_pool.tile([H + 1, FW], BF16)
            IFO = 3 * GW
            CW0 = 512                  # batch cols served by boot (tiles 0-3)
            nc.sync.dma_start(out=boot, in_=boot_d.ap())
            for j in (1, 2, 3):
                cols = bass.ts(j, GW)
                nc.sync.dma_start(out=wx[:, cols], in_=wx_d[:, cols])
                nc.sync.dma_start(out=wh1[:, cols], in_=wh1_d[:, cols])
            nc.sync.dma_start(out=c0sb[:, 0:4], in_=c0_v[:, 0:4])
            NCH = 4
            CW = (B - CW0) // NCH      # 896-col chunks for the rest
            for k in range(NCH):
                ksl = slice(CW0 + k * CW, CW0 + (k + 1) * CW)
                nc.sync.dma_start(out=xT[:, ksl], in_=xT_d[:, ksl])
                nc.sync.dma_start(out=h0T1[:, ksl], in_=h0T1_d[:, ksl])
                if k == 0:
                    nc.sync.dma_start(out=c0sb[:, 4:BT], in_=c0_v[:, 4:BT])

            # PE warm-up: dummy matmuls on a memset tile while the first
            # DMAs land. Keeps PE continuously busy from ~t=0 so the p-state
            # ramp (3us to full speed) completes before the real matmuls,
            # and the real PE queue drains in strict priority order.
            wmup = const_pool.tile([128, GW], BF16)
            nc.vector.memset(wmup, 0.0)
            wps = psum_pool.tile([128, FW], F32, name="wps", tag="psum")
            for r in range(9):
                nc.tensor.matmul(wps[:, 0:256], wmup[:, 0:128], wmup[:, 0:256],
                                 start=True, stop=True)

            def mm_operands(bt, j):
                # (lhsT_x, rhs_x, lhsT_h, rhs_h) for gate group j of tile bt;
                # tiles 0-3 and the i gate group read from the boot tile.
                rows = bass.ts(bt, 128)
                if bt < 4:
                    lx = boot[:, bt * 128:(bt + 1) * 128]
                    lh = boot[0:H + 1, 2 * GW + bt * 128:2 * GW + (bt + 1) * 128]
                else:
                    lx = xT[:, rows]
                    lh = h0T1[:, rows]
                if j == 0:
                    return lx, boot[:, GW:2 * GW], lh, boot[0:H + 1, 3 * GW:FW]
                cols = bass.ts(j, GW)
                return lx, wx[:, cols], lh, wh1[:, cols]

            QN = 4                       # tiles per c'-tanh / output quad
            osd = {}                     # bt -> Os tile (To+1)
            c2qd = {}                    # q -> c2 quad tile
            hsd = {}                     # q -> output staging tile

            def head(bt, qn=QN):
                psum = psum_pool.tile([128, FW], F32, name=f"ps{bt}", tag="psum")
                # sact = [Ti | Tf | To | Tg]: one tanh over ALL gates
                # (i,f,o pre-scaled x0.5 host-side; sig = (T+1)/2). Tiles
                # 0-1 interleave matmul-pair + per-gate tanh EMISSION so the
                # tanh's coalesced wait only covers that gate's matmuls and
                # ACT starts before the later weight DMAs land.
                sact = work.tile([128, FW], BF16, name=f"sa{bt}", tag="sact")
                if bt < 2:
                    for j in range(4):
                        cols = bass.ts(j, GW)
                        lx, rx, lh, rh = mm_operands(bt, j)
                        nc.tensor.matmul(psum[:, cols], lx, rx,
                                         start=True, stop=False)
                        nc.tensor.matmul(psum[:, cols], lh, rh,
                                         start=False, stop=True)
                        nc.scalar.activation(sact[:, cols], psum[:, cols],
                                             AF.Tanh)
                else:
                    for j in range(4):
                        cols = bass.ts(j, GW)
                        lx, rx, lh, rh = mm_operands(bt, j)
                        nc.tensor.matmul(psum[:, cols], lx, rx,
                                         start=True, stop=False)
                        nc.tensor.matmul(psum[:, cols], lh, rh,
                                         start=False, stop=True)
                    nc.scalar.activation(sact, psum, AF.Tanh)

                # c2 = 2*c' = (Tf+1)*c0 + (Ti+1)*Tg. Shifts via 4x-mode
                # tensor_scalar adds; products + final add via 2x-mode
                # tensor_tensor, all on DVE (short latency to the tail's
                # tanh); the o-gate shift on Pool (keeps DVE slack).
                is_ = work.tile([128, GW], BF16, name=f"is{bt}", tag="is")
                nc.vector.tensor_scalar_add(is_, sact[:, 0:GW], 1.0)
                m1 = work.tile([128, GW], BF16, name=f"m1{bt}", tag="m1")
                nc.vector.tensor_tensor(m1, is_, sact[:, 3 * GW:FW], ALU.mult)
                c0bc = c0sb[:, bt].unsqueeze(1).broadcast_to([128, EPC, H])
                fs = work.tile([128, GW], BF16, name=f"fs{bt}", tag="fs")
                nc.vector.tensor_scalar_add(fs, sact[:, GW:2 * GW], 1.0)
                m2 = work.tile([128, EPC, H], BF16, name=f"m2{bt}", tag="m2")
                nc.vector.tensor_tensor(
                    m2, fs.rearrange("p (e h) -> p e h", e=EPC), c0bc, ALU.mult)
                if bt % QN == 0:
                    c2qd[bt // QN] = work.tile([128, QN, GW], BF16,
                                               name=f"c2q{bt // QN}", tag="c2q")
                os_ = work.tile([128, GW], BF16, name=f"os{bt}", tag="os",
                                bufs=10)
                if bt == BT - 1:
                    # Last tile: c2 add LAST so the tail tanh starts asap.
                    nc.vector.tensor_scalar_add(os_, sact[:, 2 * GW:3 * GW], 1.0)
                    nc.vector.tensor_add(c2qd[bt // QN][:, bt % QN],
                                         m1, m2.rearrange("p e h -> p (e h)"))
                else:
                    nc.vector.tensor_add(c2qd[bt // QN][:, bt % QN],
                                         m1, m2.rearrange("p e h -> p (e h)"))
                    nc.vector.tensor_scalar_add(os_, sact[:, 2 * GW:3 * GW], 1.0)
                osd[bt] = os_

            out_v = out_d.ap().rearrange("(u p) c -> p u c", p=128)

            def tail(q, j0, nt, flush, h2_engine=None):
                # tc = tanh(c') for nt tiles of quad q (ACT input scale
                # halves c2); h2 = (To+1)*tanh(c') = 2h, stored bf16. Host
                # applies the final x0.5 in fp32.
                c2q = c2qd[q]
                tcq = work.tile([128, nt, GW], BF16, name=f"tc{q}_{j0}",
                                tag="tcq", bufs=2)
                nc.scalar.activation(tcq, c2q[:, j0:j0 + nt], AF.Tanh,
                                     scale=0.5)
                if j0 == 0:
                    hsd[q] = ostage.tile([128, QN, GW], BF16, name=f"hs{q}",
                                         tag="hs")
                hs = hsd[q]
                eng = h2_engine or nc.vector
                for j in range(j0, j0 + nt):
                    eng.tensor_tensor(hs[:, j], osd.pop(q * QN + j),
                                      tcq[:, j - j0], ALU.mult)
                if flush:
                    nc.sync.dma_start(
                        out=out_v[:, q * QN + j0:q * QN + j0 + nt],
                        in_=hs[:, j0:j0 + nt])

            NQ = BT // QN
            for bt in range(BT):
                head(bt)
                q, r = bt // QN, bt % QN
                if r == 3 and q >= 1:
                    tail(q - 1, 0, QN, flush=True)      # full quads 0..NQ-2
            # Drain: last quad split so the final ACT op and DMA are small;
            # tile 30's h2 on Pool so DVE clears tile 31's c2 chain sooner.
            tail(NQ - 1, 0, 2, flush=True)               # tiles 28,29
            tail(NQ - 1, 2, 1, flush=True, h2_engine=nc.gpsimd)  # tile 30
            tail(NQ - 1, 3, 1, flush=True)               # tile 31

    nc.compile()
    return nc


def _prep_in_maps(x, h0, c0, W_ih, W_hh, b_ih, b_hh):
    import ml_dtypes

    BF = ml_dtypes.bfloat16
    x = np.asarray(x, np.float32)
    h0 = np.asarray(h0, np.float32)
    c0 = np.asarray(c0, np.float32)
    W_ih = np.asarray(W_ih, np.float32)
    W_hh = np.asarray(W_hh, np.float32)
    b_ih = np.asarray(b_ih, np.float32)
    b_hh = np.asarray(b_hh, np.float32)

    xT = np.ascontiguousarray(x.T).astype(BF)                         # [128, B]
    h0T1 = np.concatenate([h0.T, np.ones((1, B), np.float32)], 0).astype(BF)
    c0b = np.ascontiguousarray(c0).astype(BF)                         # [B, 64]

    Wg = W_ih.reshape(N, 4, H, D)[:, _GATE_ORDER]                     # [n,t,h,d]
    Hg = W_hh.reshape(N, 4, H, H)[:, _GATE_ORDER]                     # [n,t,h,k]
    bg = (b_ih + b_hh).reshape(N, 4, H)[:, _GATE_ORDER]               # [n,t,h]

    in_maps = []
    for c in range(NCORES):
        sl = slice(c * EPC, (c + 1) * EPC)
        wx = Wg[sl].transpose(3, 1, 0, 2).reshape(D, FW).copy()       # [d, t*e*h]
        wh = Hg[sl].transpose(3, 1, 0, 2).reshape(H, FW)
        bias = bg[sl].transpose(1, 0, 2).reshape(1, FW)
        wh1 = np.concatenate([wh, bias], 0)                           # [65, 2048]
        # Pre-scale i,f,o gate columns (incl bias row) by 0.5 — exact in
        # bf16 — so ONE tanh over all gates yields sig(x) = (tanh(x/2)+1)/2.
        wx[:, 0:3 * GW] *= 0.5
        wh1[:, 0:3 * GW] *= 0.5
        wxb = np.ascontiguousarray(wx).astype(BF)
        wh1b = np.ascontiguousarray(wh1).astype(BF)
        # boot: tiles 0-3 x/h0 + the i-gate weights, one contiguous block.
        boot = np.zeros((D, FW), BF)
        boot[:, 0:GW] = xT[:, 0:GW]
        boot[:, GW:2 * GW] = wxb[:, 0:GW]
        boot[0:H + 1, 2 * GW:3 * GW] = h0T1[:, 0:GW]
        boot[0:H + 1, 3 * GW:FW] = wh1b[:, 0:GW]
        in_maps.append({
            "boot": boot,
            "xT": xT,
            "h0T1": h0T1,
            "c0": c0b,
            "wx": wxb,
            "wh1": wh1b,
        })
    return in_maps


_NC_CACHE = {}


def _run(in_maps, **kwargs):
    # Cache the built module: repeated kernel() calls then reuse both the
    # Tile-scheduled program and (via the stable nc object) the compiled
    # executable instead of rebuilding/recompiling each time.
    nc = _NC_CACHE.get("nc")
    if nc is None:
        nc = _NC_CACHE["nc"] = _build_bass()
    return run_bass_kernel_spmd(nc, in_maps, list(range(NCORES)), **kwargs)


def kernel(x, h0, c0, W_ih, W_hh, b_ih, b_hh):
    in_maps = _prep_in_maps(x, h0, c0, W_ih, W_hh, b_ih, b_hh)
    res = _run(in_maps)
    # Device output is 2h in bf16; the final x0.5 and fp32 cast are host-side.
    out = np.concatenate(
        [np.asarray(res.results[c]["out"]) for c in range(NCORES)], axis=1
    ).astype(np.float32) * 0.5
    return out, out, out
